# revision 1
# baseline (speedup 1.0000x reference)
"""LSEP loss kernel for Trainium2 (8 NeuronCores, SPMD data-parallel).

loss = log1p( sum_i [ (sum_{c: t=0} exp(x_ic)) * (sum_{c: t=1} exp(-x_ic)) ] )

Strategy: shard the batch (32768) across 8 cores (4096 rows each).
Per core, view the shard as [128 partitions, 32 samples x 1000 classes] and
stream column-chunks:
  a = x - BIG*t           (one DVE scalar_tensor_tensor op, int32 t cast on read)
  s_neg_row = sum exp(a)          -> exact exp(x) where t==0, ~0 where t==1
  s_pos_row = sum exp(-a - BIG)   -> exp(-x) where t==1, ~0 where t==0
(both exps via ScalarE activation free affine + accum_out row reduction into
PSUM accumulators). Epilogue: prod = s_neg*s_pos per sample, reduce,
DMA [128,1] partial per core; final scalar sum + log1p on host.

Chunk schedule [1,1,2,2,...]: small first chunks cut the pipeline ramp-in
(first EXP can start after 0.5 MB x2 instead of 2 MB x2).
"""

import numpy as np

BATCH = 32768
C = 1000
N_CORES = 8
ROWS = BATCH // N_CORES          # 4096 rows per core
P = 128                          # SBUF partitions
SPR = ROWS // P                  # 32 samples per partition
NSLC = SPR                       # accumulated sample-columns per partition
BIG = 50.0
# small chunks at both ends: fast pipeline ramp-in AND a short tail
# dependency chain after the last DMA completes
CHUNKS = [1, 1, 1] + [2] * 13 + [1, 1, 1]  # sum == 32

_CACHE = {}


def _build_nc():
    import concourse.bacc as bacc
    import concourse.mybir as mybir
    from concourse.tile import TileContext

    f32 = mybir.dt.float32
    i32 = mybir.dt.int32
    Exp = mybir.ActivationFunctionType.Exp
    Alu = mybir.AluOpType

    assert sum(CHUNKS) == NSLC
    wmax = max(CHUNKS) * C

    nc = bacc.Bacc()
    x = nc.declare_dram_parameter("input", [ROWS, C], f32, isOutput=False)
    t = nc.declare_dram_parameter("target", [ROWS, C], i32, isOutput=False)
    out = nc.declare_dram_parameter("partial", [P, 1], f32, isOutput=True)

    # partition p holds samples [p*32, (p+1)*32), 32000 contiguous floats
    xv = x.rearrange("(p s) c -> p (s c)", p=P)
    tv = t.rearrange("(p s) c -> p (s c)", p=P)

    with TileContext(nc) as tc:
        with (
            tc.tile_pool(name="io", bufs=4) as io,
            tc.tile_pool(name="acc", bufs=1) as accp,
            tc.tile_pool(name="ps", bufs=1, space="PSUM") as psp,
        ):
            sn = psp.tile([P, NSLC], f32)
            sp = psp.tile([P, NSLC], f32)
            escr = psp.tile([P, C], f32)  # ACT main output scratch (discarded)
            bneg = accp.tile([P, 1], f32)  # bias AP holding -BIG
            nc.vector.memset(bneg[:], -BIG)
            off = 0
            for ncols in CHUNKS:
                w = ncols * C
                xt = io.tile([P, wmax], f32, tag="x")
                tt = io.tile([P, wmax], i32, tag="t")
                at = io.tile([P, wmax], f32, tag="a")
                nc.sync.dma_start(xt[:, :w], xv[:, off * C : off * C + w])
                nc.sync.dma_start(tt[:, :w], tv[:, off * C : off * C + w])
                # a = (t * -BIG) + x
                nc.vector.scalar_tensor_tensor(
                    at[:, :w], tt[:, :w], -BIG, xt[:, :w],
                    op0=Alu.mult, op1=Alu.add,
                )
                for j in range(ncols):
                    k = off + j
                    seg = at[:, j * C : (j + 1) * C]
                    # s_neg: exp(a); masked (t==1) entries exp(x-50) ~ 0
                    nc.scalar.activation(
                        escr[:], seg, Exp, accum_out=sn[:, k : k + 1]
                    )
                    # s_pos: exp(-a-50); masked (t==0) entries exp(-x-50) ~ 0
                    nc.scalar.activation(
                        escr[:], seg, Exp, scale=-1.0, bias=bneg[:],
                        accum_out=sp[:, k : k + 1],
                    )
                off += ncols
            # epilogue: prod per sample-column, reduce, write [128,1] partial
            sns = accp.tile([P, NSLC], f32)
            prod = accp.tile([P, NSLC], f32)
            tot = accp.tile([P, 1], f32)
            nc.vector.tensor_copy(sns[:], sn[:])
            nc.vector.tensor_tensor(prod[:], sns[:], sp[:], Alu.mult)
            nc.vector.reduce_sum(tot[:], prod[:], axis=mybir.AxisListType.X)
            # out-DMA on the ACT HWDGE ring: the sync ring's FIFO still
            # holds input-DMA completions at this point
            nc.scalar.dma_start(out[:], tot[:])
    # Bacc.compile() legalizes sync waits (ISA allows 1 wait/instruction;
    # extra waits become standalone EventSemaphore instructions).
    nc.compile()
    return nc


def _get_nc():
    if "nc" not in _CACHE:
        _CACHE["nc"] = _build_nc()
    return _CACHE["nc"]


def kernel(input, target):
    from concourse.bass_utils import run_bass_kernel_spmd

    x = np.ascontiguousarray(np.asarray(input, dtype=np.float32))
    t = np.ascontiguousarray(np.asarray(target, dtype=np.int32))
    assert x.shape == (BATCH, C) and t.shape == (BATCH, C)

    nc = _get_nc()
    in_maps = [
        {
            "input": x[i * ROWS : (i + 1) * ROWS],
            "target": t[i * ROWS : (i + 1) * ROWS],
        }
        for i in range(N_CORES)
    ]
    res = run_bass_kernel_spmd(nc, in_maps, list(range(N_CORES)))
    total = 0.0
    for r in res.results:
        total += float(np.sum(r["partial"].astype(np.float64)))
    return np.asarray([np.log1p(total)], dtype=np.float32)



# revision 2
# speedup vs baseline: 1.0369x; 1.0369x over previous
"""LSEP loss kernel for Trainium2 (8 NeuronCores, SPMD data-parallel).

loss = log1p( sum_i [ (sum_{c: t=0} exp(x_ic)) * (sum_{c: t=1} exp(-x_ic)) ] )

Strategy: shard the batch (32768) across 8 cores (4096 rows each). On the
host, pack each core's x (f32 bits) and t (i32) shards into one interleaved
[4096, 2000] i32 tensor (row r = [x_r | t_r]) so every chunk needs a single
DMA and x/t land together. Per core, view the shard as [128 partitions,
32 samples, 2000] and stream sample-column chunks:

  a  = x - 50*t                       (one DVE scalar_tensor_tensor)
  s_neg[k] = sum exp(a)               per column: ACT EXP with accum_out
                                      (masked (t==1) entries exp(x-50) ~ 0)
  e  = exp(-a - 50)                   one wide ACT EXP per chunk
                                      (masked (t==0) entries exp(-x-50) ~ 0)
  s_pos[k] = sum_c e                  DVE grouped reduce_sum (axis X)

ACT per 2-col chunk: 2x accum-EXP (N=1000) + 1x wide EXP (N=2000) = 4.46us;
DVE: stt (2.24us) + grouped reduce (2.24us) -- both under the ~5.6us DMA
cadence, so the HBM stream (~32.8 MB/core at ~360-400 GB/s) is the limiter.

Scheduling details:
  - ACT emits the wide EXP first so the DVE reduce doesn't wait for the
    accum-EXPs; the DVE reduce of chunk N is emitted after the stt of
    chunk N+1 (DVE executes in order -- this keeps stt off the ACT path).
  - Epilogue reduces to a single scalar on-chip (DVE product+reduce, then
    a PE ones-matmul across partitions) so the output DMA is one 4-byte
    descriptor instead of 128 (which each cost an HBM read-modify-write).
  - Small chunks at both ends: fast pipeline ramp-in and a short tail.
"""

import numpy as np

BATCH = 32768
C = 1000
N_CORES = 8
ROWS = BATCH // N_CORES          # 4096 rows per core
P = 128                          # SBUF partitions
SPR = ROWS // P                  # 32 samples per partition
NSLC = SPR
BIG = 50.0
CHUNKS = [1, 1, 1, 1] + [2] * 13 + [1, 1]  # sum == 32
MAXC = max(CHUNKS)

_CACHE = {}


def _build_nc():
    import concourse.bacc as bacc
    import concourse.mybir as mybir
    from concourse.tile import TileContext

    f32 = mybir.dt.float32
    i32 = mybir.dt.int32
    Exp = mybir.ActivationFunctionType.Exp
    Alu = mybir.AluOpType
    X = mybir.AxisListType.X

    assert sum(CHUNKS) == NSLC

    nc = bacc.Bacc()
    xt = nc.declare_dram_parameter("xt", [ROWS, 2 * C], i32, isOutput=False)
    out = nc.declare_dram_parameter("partial", [1, 1], f32, isOutput=True)

    # partition p holds samples [p*32, (p+1)*32); each sample row is
    # [1000 x-words | 1000 t-words]
    xtv = xt.rearrange("(p s) c -> p s c", p=P)

    with TileContext(nc) as tc:
        with (
            tc.tile_pool(name="io", bufs=4) as io,
            tc.tile_pool(name="acc", bufs=1) as accp,
            tc.tile_pool(name="ps", bufs=1, space="PSUM") as psp,
        ):
            sn = psp.tile([P, NSLC], f32)     # s_neg accumulators
            escr = psp.tile([P, C], f32)      # accum-EXP main out (discarded)
            pe1 = psp.tile([1, 1], f32)
            bneg = accp.tile([P, 1], f32)     # bias AP holding -BIG
            ones = accp.tile([P, 1], f32)
            sp_all = accp.tile([P, NSLC], f32)
            nc.vector.memset(bneg[:], -BIG)
            nc.vector.memset(ones[:], 1.0)

            pending = None  # (e_tile, ncols, k) whose reduce is deferred
            off = 0
            for ncols in CHUNKS:
                xtt = io.tile([P, MAXC, 2 * C], i32, tag="xt")
                at = io.tile([P, MAXC, C], f32, tag="a")
                et = io.tile([P, MAXC, C], f32, tag="e")
                nc.sync.dma_start(
                    xtt[:, :ncols, :], xtv[:, off : off + ncols, :]
                )
                # a = (t * -BIG) + x   (x = low half bit-cast back to f32)
                nc.vector.scalar_tensor_tensor(
                    at[:, :ncols, :],
                    xtt[:, :ncols, C:],
                    -BIG,
                    xtt[:, :ncols, :C].bitcast(f32),
                    op0=Alu.mult,
                    op1=Alu.add,
                )
                # deferred reduce of the PREVIOUS chunk (keeps DVE's in-order
                # queue from putting this between ACT and the next stt)
                if pending is not None:
                    pe, pn, pk = pending
                    nc.vector.reduce_sum(
                        sp_all[:, pk : pk + pn], pe[:, :pn, :], axis=X
                    )
                # s_pos elementwise: exp(-a - BIG), one wide EXP (emitted
                # before the accum-EXPs so the reduce isn't gated on them)
                nc.scalar.activation(
                    et[:, :ncols, :], at[:, :ncols, :], Exp,
                    scale=-1.0, bias=bneg[:],
                )
                # s_neg: per-column EXP with row-sum accumulator
                for j in range(ncols):
                    nc.scalar.activation(
                        escr[:], at[:, j, :], Exp,
                        accum_out=sn[:, off + j : off + j + 1],
                    )
                pending = (et, ncols, off)
                off += ncols
            pe, pn, pk = pending
            nc.vector.reduce_sum(sp_all[:, pk : pk + pn], pe[:, :pn, :], axis=X)

            # epilogue: per-sample product, reduce to [P,1], then collapse
            # partitions with a ones-matmul -> single-scalar output DMA
            prod = accp.tile([P, NSLC], f32)
            tot = accp.tile([P, 1], f32)
            res = accp.tile([1, 1], f32)
            nc.vector.tensor_tensor(prod[:], sn[:], sp_all[:], Alu.mult)
            nc.vector.reduce_sum(tot[:], prod[:], axis=X)
            nc.tensor.matmul(pe1[:], ones[:], tot[:])
            nc.vector.tensor_copy(res[:], pe1[:])
            # out-DMA on the ACT HWDGE ring: the sync ring's FIFO still
            # holds input-DMA completions at this point
            nc.scalar.dma_start(out[:], res[:])
    nc.compile()
    return nc


def _get_nc():
    if "nc" not in _CACHE:
        _CACHE["nc"] = _build_nc()
    return _CACHE["nc"]


def make_in_maps(x, t):
    """Pack per-core shards: [ROWS, 2000] i32 = [x bits | t] per row."""
    x = np.ascontiguousarray(np.asarray(x, dtype=np.float32))
    t = np.ascontiguousarray(np.asarray(t, dtype=np.int32))
    assert x.shape == (BATCH, C) and t.shape == (BATCH, C)
    in_maps = []
    for i in range(N_CORES):
        comb = np.empty((ROWS, 2 * C), dtype=np.int32)
        comb[:, :C] = x[i * ROWS : (i + 1) * ROWS].view(np.int32)
        comb[:, C:] = t[i * ROWS : (i + 1) * ROWS]
        in_maps.append({"xt": comb})
    return in_maps


def kernel(input, target):
    from concourse.bass_utils import run_bass_kernel_spmd

    nc = _get_nc()
    in_maps = make_in_maps(input, target)
    res = run_bass_kernel_spmd(nc, in_maps, list(range(N_CORES)))
    total = 0.0
    for r in res.results:
        total += float(r["partial"][0, 0])
    return np.asarray([np.log1p(total)], dtype=np.float32)


# revision 3
# speedup vs baseline: 1.0681x; 1.0301x over previous
"""LSEP loss kernel for Trainium2 (8 NeuronCores, SPMD data-parallel).

loss = log1p( sum_i [ (sum_{c: t=0} exp(x_ic)) * (sum_{c: t=1} exp(-x_ic)) ] )

Strategy: shard the batch (32768) across 8 cores (4096 rows each). On the
host, pack each core's x (f32 bits) and t (i32) shards into one interleaved
[4096, 2000] i32 tensor (row r = [x_r | t_r]) so every chunk needs a single
DMA and x/t land together. Per core, view the shard as [128 partitions,
32 samples, 2000] and stream sample-column chunks:

  a  = x - 50*t                       (one DVE scalar_tensor_tensor)
  s_neg[k] = sum exp(a)               per column: ACT EXP with accum_out
                                      (masked (t==1) entries exp(x-50) ~ 0)
  e  = exp(-a - 50)                   one wide ACT EXP per chunk
                                      (masked (t==0) entries exp(-x-50) ~ 0)
  s_pos[k] = sum_c e                  DVE grouped reduce_sum (axis X)

ACT per 2-col chunk: 2x accum-EXP (N=1000) + 1x wide EXP (N=2000) = 4.46us;
DVE: stt (2.24us) + grouped reduce (2.24us) -- both under the ~5.6us DMA
cadence, so the HBM stream (~32.8 MB/core at ~360-400 GB/s) is the limiter.

Scheduling details:
  - ACT emits the wide EXP first so the DVE reduce doesn't wait for the
    accum-EXPs; the DVE reduce of chunk N is emitted after the stt of
    chunk N+1 (DVE executes in order -- this keeps stt off the ACT path).
  - Epilogue reduces to a single scalar on-chip (DVE product+reduce, then
    a PE ones-matmul across partitions) so the output DMA is one 4-byte
    descriptor instead of 128 (which each cost an HBM read-modify-write).
  - Small chunks at both ends: fast pipeline ramp-in and a short tail.
"""

import numpy as np

BATCH = 32768
C = 1000
N_CORES = 8
ROWS = BATCH // N_CORES          # 4096 rows per core
P = 128                          # SBUF partitions
SPR = ROWS // P                  # 32 samples per partition
NSLC = SPR
BIG = 50.0
CHUNKS = [1, 1, 1, 1] + [2] * 13 + [1, 1]  # sum == 32
MAXC = max(CHUNKS)

_CACHE = {}


def _build_nc():
    import concourse.bacc as bacc
    import concourse.mybir as mybir
    from concourse.tile import TileContext

    f32 = mybir.dt.float32
    i32 = mybir.dt.int32
    Exp = mybir.ActivationFunctionType.Exp
    Alu = mybir.AluOpType
    X = mybir.AxisListType.X

    assert sum(CHUNKS) == NSLC

    nc = bacc.Bacc()
    xt = nc.declare_dram_parameter("xt", [ROWS, 2 * C], i32, isOutput=False)
    out = nc.declare_dram_parameter("partial", [1, 1], f32, isOutput=True)

    # partition p holds samples [p*32, (p+1)*32); each sample row is
    # [1000 x-words | 1000 t-words]
    xtv = xt.rearrange("(p s) c -> p s c", p=P)

    with TileContext(nc) as tc:
        with (
            tc.tile_pool(name="xtp", bufs=5) as xtp,
            tc.tile_pool(name="ap", bufs=4) as apool,
            tc.tile_pool(name="ep", bufs=4) as epool,
            tc.tile_pool(name="acc", bufs=1) as accp,
            tc.tile_pool(name="ps", bufs=1, space="PSUM") as psp,
        ):
            sn = psp.tile([P, NSLC], f32)     # s_neg accumulators
            escr = psp.tile([P, C], f32)      # accum-EXP main out (discarded)
            pe1 = psp.tile([1, 1], f32)
            bneg = accp.tile([P, 1], f32)     # bias AP holding -BIG
            ones = accp.tile([P, 1], f32)
            sp_all = accp.tile([P, NSLC], f32)
            nc.vector.memset(bneg[:], -BIG)
            nc.vector.memset(ones[:], 1.0)

            # s_pos reduces run LAG chunks late so the DVE in-order queue
            # never puts a reduce (gated on ACT) in front of an stt the ACT
            # engine is about to need; the per-iteration tile_set_cur_wait
            # floor stops the tile scheduler from hoisting them back.
            LAG = 2
            pending = []  # [(e_tile, ncols, k)] reduces not yet emitted
            off = 0
            for it, ncols in enumerate(CHUNKS):
                tc.tile_set_cur_wait(0.02 * (it + 1))
                xtt = xtp.tile([P, MAXC, 2 * C], i32, tag="xt")
                at = apool.tile([P, MAXC, C], f32, tag="a")
                et = epool.tile([P, MAXC, C], f32, tag="e")
                nc.sync.dma_start(
                    xtt[:, :ncols, :], xtv[:, off : off + ncols, :]
                )
                # a = (t * -BIG) + x   (x = low half bit-cast back to f32)
                nc.vector.scalar_tensor_tensor(
                    at[:, :ncols, :],
                    xtt[:, :ncols, C:],
                    -BIG,
                    xtt[:, :ncols, :C].bitcast(f32),
                    op0=Alu.mult,
                    op1=Alu.add,
                )
                if len(pending) >= LAG:
                    pe, pn, pk = pending.pop(0)
                    nc.vector.reduce_sum(
                        sp_all[:, pk : pk + pn], pe[:, :pn, :], axis=X
                    )
                # s_pos elementwise: exp(-a - BIG), one wide EXP (emitted
                # before the accum-EXPs so the reduce isn't gated on them)
                nc.scalar.activation(
                    et[:, :ncols, :], at[:, :ncols, :], Exp,
                    scale=-1.0, bias=bneg[:],
                )
                # s_neg: per-column EXP with row-sum accumulator
                for j in range(ncols):
                    nc.scalar.activation(
                        escr[:], at[:, j, :], Exp,
                        accum_out=sn[:, off + j : off + j + 1],
                    )
                pending.append((et, ncols, off))
                off += ncols
            for i, (pe, pn, pk) in enumerate(pending):
                tc.tile_set_cur_wait(0.02 * (len(CHUNKS) + 1 + i))
                nc.vector.reduce_sum(
                    sp_all[:, pk : pk + pn], pe[:, :pn, :], axis=X
                )
            tc.tile_set_cur_wait(0.02 * (len(CHUNKS) + 4))

            # epilogue: per-sample product, reduce to [P,1], then collapse
            # partitions with a ones-matmul -> single-scalar output DMA
            prod = accp.tile([P, NSLC], f32)
            tot = accp.tile([P, 1], f32)
            res = accp.tile([1, 1], f32)
            nc.vector.tensor_tensor(prod[:], sn[:], sp_all[:], Alu.mult)
            nc.vector.reduce_sum(tot[:], prod[:], axis=X)
            nc.tensor.matmul(pe1[:], ones[:], tot[:])
            nc.vector.tensor_copy(res[:], pe1[:])
            # out-DMA on the ACT HWDGE ring: the sync ring's FIFO still
            # holds input-DMA completions at this point
            nc.scalar.dma_start(out[:], res[:])
    nc.compile()
    return nc


def _get_nc():
    if "nc" not in _CACHE:
        _CACHE["nc"] = _build_nc()
    return _CACHE["nc"]


def make_in_maps(x, t):
    """Pack per-core shards: [ROWS, 2000] i32 = [x bits | t] per row."""
    x = np.ascontiguousarray(np.asarray(x, dtype=np.float32))
    t = np.ascontiguousarray(np.asarray(t, dtype=np.int32))
    assert x.shape == (BATCH, C) and t.shape == (BATCH, C)
    in_maps = []
    for i in range(N_CORES):
        comb = np.empty((ROWS, 2 * C), dtype=np.int32)
        comb[:, :C] = x[i * ROWS : (i + 1) * ROWS].view(np.int32)
        comb[:, C:] = t[i * ROWS : (i + 1) * ROWS]
        in_maps.append({"xt": comb})
    return in_maps


def kernel(input, target):
    from concourse.bass_utils import run_bass_kernel_spmd

    nc = _get_nc()
    in_maps = make_in_maps(input, target)
    res = run_bass_kernel_spmd(nc, in_maps, list(range(N_CORES)))
    total = 0.0
    for r in res.results:
        total += float(r["partial"][0, 0])
    return np.asarray([np.log1p(total)], dtype=np.float32)
